# revision 1
# baseline (speedup 1.0000x reference)
"""Multi-head self-attention on 8 Trainium2 NeuronCores.

Problem: B=4, S=2048, D=1024, H=16 heads (head_dim 64), fp32.
  out = softmax((x Wq + bq)(x Wk + bk)^T / 8) (x Wv + bv) Wo + bo

Sharding: 8 shards = 4 batches x 2 head-groups (8 heads each).
Core c handles batch c//2, heads (c%2)*8 .. (c%2)*8+8.  Wq/Wk/Wv are
column-sharded, Wo row-sharded; each core emits a partial [S, D] output
and the host sums the two partials per batch (the Wo all-reduce) + bo.

Per-core dataflow (matmuls in float32r: full-rate fp32, ~1e-3 rel err):
  x^T (host-pretransposed [D, S]) lives in SBUF.
  Q^T[dg,s], K^T[dg,s]: weight-stationary matmuls (lhsT=Wq chunk, rhs=x^T).
  V[s,dg]: x-stationary matmuls (lhsT=x^T chunk, rhs=Wv), stored with a
    ones column per head -> the PV matmul also produces the softmax sums.
  Attention per (head, si-block): logits^T[sj,si] = (K^T chunk)^T Q^T,
  exp on ScalarE (scale=1/8; no max subtraction: logits ~ N(0,1)),
  P^T V via lhsT=[V|1] -> vals^T[hd,si] + sums[si] in one accumulation,
  normalize: reciprocal(sums), gpsimd partition-broadcast, multiply.
  vals^T is exactly the lhsT layout the output projection needs.
  K^T is spilled to DRAM after projection (SBUF pressure) and streamed
  back per head pair.
"""
import numpy as np

B, S, D, H = 4, 2048, 1024, 16
HD = D // H          # 64
G = D // 2           # 512 columns per head-group
NCORES = 8
KT_ = 8              # D / 128 contraction tiles
TT = 4               # G / 128 dg tiles
ST = 16              # S / 128 s tiles
SB = 2               # si blocks
SBW = 1024           # si block width

_cache = {}


def _split_sync_waits(nc, mybir, max_waits=1):
    """walrus on this toolchain rejects >1 sem wait per instruction; move
    extra waits onto same-engine NoOps placed just before the instruction
    (engines are in-order, so this is semantics-preserving)."""
    for f in nc.m.functions:
        for bb in f.blocks:
            out, changed = [], False
            for inst in bb.instructions:
                si = inst.sync_info
                if si is not None and len(si.on_wait) > max_waits:
                    waits = list(si.on_wait)
                    head, tail = waits[:-max_waits], waits[-max_waits:]
                    for g in range(0, len(head), max_waits):
                        nop = mybir.InstNoOp(name=nc.get_next_instruction_name())
                        nop.engine = inst.engine
                        nop.sync_info = mybir.SyncInfo(
                            on_wait=head[g:g + max_waits], on_update=[])
                        nc.register_instruction(nop)
                        out.append(nop)
                    inst.sync_info = mybir.SyncInfo(
                        on_wait=tail, on_update=list(si.on_update))
                    changed = True
                out.append(inst)
            if changed:
                bb.instructions = out


def _build():
    import concourse.bass as bass
    import concourse.mybir as mybir
    import concourse.tile as tile

    F32 = mybir.dt.float32
    FR = mybir.dt.float32r
    Exp = mybir.ActivationFunctionType.Exp

    nc = bass.Bass("TRN2", target_bir_lowering=False, debug=False,
                   num_devices=NCORES)
    xtd = nc.dram_tensor("xt", [D, S], FR, kind="ExternalInput")
    wqd = nc.dram_tensor("wq", [D, G], FR, kind="ExternalInput")
    wkd = nc.dram_tensor("wk", [D, G], FR, kind="ExternalInput")
    wvd = nc.dram_tensor("wv", [D, G], FR, kind="ExternalInput")
    wod = nc.dram_tensor("wo", [G, D], FR, kind="ExternalInput")
    bqd = nc.dram_tensor("bq", [G], F32, kind="ExternalInput")
    bkd = nc.dram_tensor("bk", [G], F32, kind="ExternalInput")
    bvd = nc.dram_tensor("bv", [G], F32, kind="ExternalInput")
    onesd = nc.dram_tensor("ones", [ST, 8], FR, kind="ExternalInput")
    outd = nc.dram_tensor("out", [S, D], F32, kind="ExternalOutput")

    with tile.TileContext(nc) as tc:
        with tc.tile_pool(name="persist", bufs=1) as pp, \
             tc.tile_pool(name="dram", bufs=1, space="DRAM") as dp:
            qts = pp.tile([128, TT, S], FR, tag="qts")
            vsb = pp.tile([128, ST, 8, HD + 1], FR, tag="vsb")
            bqt = pp.tile([128, TT], F32, tag="bqt")
            bkt = pp.tile([128, TT], F32, tag="bkt")
            bvt = pp.tile([64, 8], F32, tag="bvt")
            ktd = dp.tile([G, S], FR, tag="ktd")

            nc.sync.dma_start(out=bqt, in_=bqd.rearrange("(t p) -> p t", p=128))
            nc.sync.dma_start(out=bkt, in_=bkd.rearrange("(t p) -> p t", p=128))
            nc.sync.dma_start(out=bvt, in_=bvd.rearrange("(h p) -> p h", p=64))
            nc.sync.dma_start(
                out=vsb[:, :, :, HD:HD + 1],
                in_=onesd[:, :].partition_broadcast(128))

            # ---- Phase 1: projections ----
            with tc.tile_pool(name="proj", bufs=1) as jp, \
                 tc.tile_pool(name="stage", bufs=3) as sp, \
                 tc.tile_pool(name="ps_proj", bufs=5, space="PSUM") as psp:
                xts = jp.tile([128, KT_, S], FR, tag="xts")
                wqs = jp.tile([128, KT_, G], FR, tag="wqs")
                wks = jp.tile([128, KT_, G], FR, tag="wks")
                wvs = jp.tile([128, KT_, G], FR, tag="wvs")
                for k in range(KT_):
                    nc.sync.dma_start(out=wqs[:, k, :], in_=wqd[k * 128:(k + 1) * 128, :])
                    nc.sync.dma_start(out=wks[:, k, :], in_=wkd[k * 128:(k + 1) * 128, :])
                for k in range(KT_):
                    nc.sync.dma_start(out=xts[:, k, :], in_=xtd[k * 128:(k + 1) * 128, :])
                for k in range(KT_):
                    nc.sync.dma_start(out=wvs[:, k, :], in_=wvd[k * 128:(k + 1) * 128, :])

                # Q^T and K^T: weight-stationary over 4 dg tiles
                for which, ws, bt in (("q", wqs, bqt), ("k", wks, bkt)):
                    for t in range(TT):
                        pss = [psp.tile([128, 512], F32, tag="pj", name="pj")
                               for _ in range(4)]
                        for k in range(KT_):
                            for sc in range(4):
                                nc.tensor.matmul(
                                    pss[sc],
                                    ws[:, k, t * 128:(t + 1) * 128],
                                    xts[:, k, sc * 512:(sc + 1) * 512],
                                    start=(k == 0), stop=(k == KT_ - 1))
                        for sc in range(4):
                            if which == "q":
                                nc.vector.tensor_scalar_add(
                                    qts[:, t, sc * 512:(sc + 1) * 512],
                                    pss[sc], bt[:, t:t + 1])
                            else:
                                st = sp.tile([128, 512], FR, tag="kst")
                                nc.vector.tensor_scalar_add(st, pss[sc], bt[:, t:t + 1])
                                nc.gpsimd.dma_start(
                                    out=ktd[t * 128:(t + 1) * 128,
                                            sc * 512:(sc + 1) * 512],
                                    in_=st)

                # V: x-stationary
                for s_ in range(ST):
                    ps = psp.tile([128, 512], F32, tag="pj", name="pj")
                    for k in range(KT_):
                        nc.tensor.matmul(
                            ps, xts[:, k, s_ * 128:(s_ + 1) * 128],
                            wvs[:, k, :],
                            start=(k == 0), stop=(k == KT_ - 1))
                    nc.vector.tensor_copy(
                        out=vsb[:, s_, :, 0:HD],
                        in_=ps.rearrange("p (h d) -> p h d", h=8))

            # ---- Phase 2: attention ----
            with tc.tile_pool(name="att", bufs=1) as ap, \
                 tc.tile_pool(name="ktp", bufs=2) as ktp, \
                 tc.tile_pool(name="ppool", bufs=3) as ppl, \
                 tc.tile_pool(name="bcp", bufs=2) as bcp, \
                 tc.tile_pool(name="rcp", bufs=2) as rcp, \
                 tc.tile_pool(name="outp", bufs=3) as op_, \
                 tc.tile_pool(name="ps_big", bufs=2, space="PSUM") as psb, \
                 tc.tile_pool(name="ps_pv", bufs=2, space="PSUM") as pspv:
                valsn = ap.tile([128, TT, S], FR, tag="valsn")
                wos = ap.tile([128, TT, D], FR, tag="wos")
                for t in range(TT):
                    nc.sync.dma_start(out=wos[:, t, :], in_=wod[t * 128:(t + 1) * 128, :])

                for t in range(TT):
                    # head pair (2t, 2t+1); lhsT/rhs partition bases must match,
                    # so keep both heads' K^T and Q^T at their natural halves.
                    kt2 = ktp.tile([128, S], FR, tag="kt2")
                    nc.sync.dma_start(out=kt2, in_=ktd[t * 128:(t + 1) * 128, :])
                    for hh in range(2):
                        h = 2 * t + hh
                        p0 = hh * 64
                        qrow = qts[p0:p0 + 64, t, :]
                        for b in range(SB):
                            pv = pspv.tile([65, SBW], F32, tag="pv")
                            for sj in range(ST):
                                lg = psb.tile([128, SBW], F32, tag="big")
                                lkt = kt2[p0:p0 + 64, sj * 128:(sj + 1) * 128]
                                for half in range(2):
                                    nc.tensor.matmul(
                                        lg[:, half * 512:(half + 1) * 512],
                                        lkt,
                                        qrow[:, b * SBW + half * 512:
                                             b * SBW + (half + 1) * 512],
                                        start=True, stop=True)
                                pt = ppl.tile([128, SBW], FR, tag="pt")
                                nc.scalar.activation(pt, lg, Exp, scale=0.125)
                                lv = vsb[:, sj, h, 0:HD + 1]
                                for half in range(2):
                                    nc.tensor.matmul(
                                        pv[:, half * 512:(half + 1) * 512],
                                        lv,
                                        pt[:, half * 512:(half + 1) * 512],
                                        start=(sj == 0), stop=(sj == ST - 1))
                            rc = rcp.tile([1, SBW], F32, tag="rc")
                            nc.vector.reciprocal(out=rc, in_=pv[64:65, :])
                            # broadcast across partitions: bounce through DRAM
                            # (DMA can replicate a DRAM source; SBUF sources
                            # need nonzero partition step)
                            rcd = dp.tile([SBW], F32, tag="rcd", bufs=3)
                            nc.gpsimd.dma_start(
                                out=rcd.rearrange("(a b) -> a b", a=1), in_=rc)
                            bc = bcp.tile([64, SBW], F32, tag="bc")
                            nc.gpsimd.dma_start(
                                out=bc,
                                in_=rcd.rearrange("(a b) -> a b", a=1)
                                       .partition_broadcast(64))
                            bvcol = bvt[0:64, h:h + 1]
                            if hh == 0:
                                vn = valsn[0:64, t, b * SBW:(b + 1) * SBW]
                                nc.vector.tensor_mul(vn, pv[0:64, :], bc)
                                nc.vector.tensor_scalar_add(vn, vn, bvcol)
                            else:
                                # DVE lanes can't shift partitions; compute at
                                # base 0 and DMA-shift into partitions 64:128.
                                vs = bcp.tile([64, SBW], FR, tag="vshift")
                                nc.vector.tensor_mul(vs, pv[0:64, :], bc)
                                nc.vector.tensor_scalar_add(vs, vs, bvcol)
                                nc.gpsimd.dma_start(
                                    out=valsn[64:128, t, b * SBW:(b + 1) * SBW],
                                    in_=vs)

                # ---- Phase 3: output projection ----
                for s_ in range(ST):
                    s0 = s_ * 128
                    ops = psb.tile([128, SBW], F32, tag="big", name="ops")
                    for t in range(TT):
                        for half in range(2):
                            nc.tensor.matmul(
                                ops[:, half * 512:(half + 1) * 512],
                                valsn[:, t, s0:s0 + 128],
                                wos[:, t, half * 512:(half + 1) * 512],
                                start=(t == 0), stop=(t == TT - 1))
                    ob = op_.tile([128, D], F32, tag="ob")
                    nc.vector.tensor_copy(out=ob, in_=ops)
                    nc.gpsimd.dma_start(out=outd[s0:s0 + 128, :], in_=ob)

    _split_sync_waits(nc, mybir)
    return nc


def _get_nc():
    if "nc" not in _cache:
        _cache["nc"] = _build()
    return _cache["nc"]


def _run(in_maps, **kw):
    from concourse.bass_utils import run_bass_kernel_spmd
    return run_bass_kernel_spmd(_get_nc(), in_maps, core_ids=list(range(NCORES)), **kw)


def _make_in_maps(x, Wq, bq, Wk, bk, Wv, bv, Wo, bo):
    x = np.asarray(x, np.float32)
    in_maps = []
    for c in range(NCORES):
        b, g = c // 2, c % 2
        gs = slice(g * G, (g + 1) * G)
        in_maps.append({
            "xt": np.ascontiguousarray(x[b].T),
            "wq": np.ascontiguousarray(np.asarray(Wq, np.float32)[:, gs]),
            "wk": np.ascontiguousarray(np.asarray(Wk, np.float32)[:, gs]),
            "wv": np.ascontiguousarray(np.asarray(Wv, np.float32)[:, gs]),
            "wo": np.ascontiguousarray(np.asarray(Wo, np.float32)[gs, :]),
            "bq": np.ascontiguousarray(np.asarray(bq, np.float32)[gs]),
            "bk": np.ascontiguousarray(np.asarray(bk, np.float32)[gs]),
            "bv": np.ascontiguousarray(np.asarray(bv, np.float32)[gs]),
            "ones": np.ones((ST, 8), np.float32),
        })
    return in_maps


def kernel(x, Wq, bq, Wk, bk, Wv, bv, Wo, bo, **_kw):
    res = _run(_make_in_maps(x, Wq, bq, Wk, bk, Wv, bv, Wo, bo))
    bo = np.asarray(bo, np.float32)
    out = np.empty((B, S, D), dtype=np.float32)
    for b in range(B):
        out[b] = res.results[2 * b]["out"] + res.results[2 * b + 1]["out"] + bo
    return out



# revision 7
# speedup vs baseline: 1.2015x; 1.2015x over previous
"""Multi-head self-attention on 8 Trainium2 NeuronCores.

Problem: B=4, S=2048, D=1024, H=16 heads (head_dim 64), fp32.
  out = softmax((x Wq + bq)(x Wk + bk)^T / 8) (x Wv + bv) Wo + bo

Sharding: 8 shards = 4 batches x 2 head-groups (8 heads each).
Core c handles batch c//2, heads (c%2)*8 .. +8.  Wq/Wk/Wv column-sharded,
Wo row-sharded; each core emits a partial [S, D] output and the host sums
the two partials per batch + (bv @ Wo + bo).

Bias algebra (exact): bk shifts every logit in a query row equally ->
softmax-invariant -> dropped.  bv contributes (bv @ Wo) to every output
row (softmax weights sum to 1) -> folded into the host-side bias add.
Only bq stays on-chip (applied to Q at projection drain).

Per-core dataflow:
  Projections in float32r (full-rate fp32): x^T resident [D, S]; K^T and
  Q^T weight-stationary (k-outer over 2-t-tile groups so PE keeps pace
  with the x DMA), V x-stationary.  All drains convert to bf16: qts/kts
  [128, 4, S], vsb [128 sj, 16 sjt, 8 h, 65] with a ones column (PV then
  also produces softmax sums), valsT [128, 4, S].
  Attention per (si-block of 1024, head): logits^T tiles [sj 128, si
  1024] = 2 matmuls (lhsT = K^T chunk), exp on ScalarE (scale 1/8, no
  max subtraction: logits ~ N(0,1)) -> P^T bf16, PV accumulates
  [65, 1024] over 16 sj tiles.  Softmax sums (row 64) bounce
  PSUM->DRAM->SBUF[128,8] so the reciprocal runs on 128 partitions
  (DVE reciprocal is ~6.4 ns/elem on one partition), then
  DRAM-partition-broadcast to [64, 1024] and a fused
  normalize-multiply drains vals^T bf16 (odd heads DMA-shift into
  partitions 64:128).  Output projection per si-tile reuses the logits
  PSUM ring; partial [S, D] fp32 DMAs out.
"""
import numpy as np

B, S, D, H = 4, 2048, 1024, 16
HD = D // H          # 64
G = D // 2           # 512 columns per head-group
NCORES = 8
KT_ = 8              # D / 128 contraction tiles
TT = 4               # G / 128 dg tiles
ST = 16              # S / 128 s tiles
SB = 2               # si blocks
SBW = 1024           # si block width

_cache = {}


def _split_sync_waits(nc, mybir, max_waits=1):
    """walrus on this toolchain rejects >1 sem wait per instruction; move
    extra waits onto same-engine NoOps placed just before the instruction
    (engines are in-order, so this is semantics-preserving)."""
    for f in nc.m.functions:
        for bb in f.blocks:
            out, changed = [], False
            for inst in bb.instructions:
                si = inst.sync_info
                if si is not None and len(si.on_wait) > max_waits:
                    waits = list(si.on_wait)
                    head, tail = waits[:-max_waits], waits[-max_waits:]
                    for g in range(0, len(head), max_waits):
                        nop = mybir.InstNoOp(name=nc.get_next_instruction_name())
                        nop.engine = inst.engine
                        nop.sync_info = mybir.SyncInfo(
                            on_wait=head[g:g + max_waits], on_update=[])
                        nc.register_instruction(nop)
                        out.append(nop)
                    inst.sync_info = mybir.SyncInfo(
                        on_wait=tail, on_update=list(si.on_update))
                    changed = True
                out.append(inst)
            if changed:
                bb.instructions = out


def _build():
    import concourse.bass as bass
    import concourse.mybir as mybir
    import concourse.tile as tile

    F32 = mybir.dt.float32
    FR = mybir.dt.float32r
    BF16 = mybir.dt.bfloat16
    Exp = mybir.ActivationFunctionType.Exp

    nc = bass.Bass("TRN2", target_bir_lowering=False, debug=False,
                   num_devices=NCORES)
    xtd = nc.dram_tensor("xt", [D, S], FR, kind="ExternalInput")
    wqd = nc.dram_tensor("wq", [D, G], FR, kind="ExternalInput")
    wkd = nc.dram_tensor("wk", [D, G], FR, kind="ExternalInput")
    wvd = nc.dram_tensor("wv", [D, G], FR, kind="ExternalInput")
    wod = nc.dram_tensor("wo", [G, D], BF16, kind="ExternalInput")
    bqd = nc.dram_tensor("bq", [G], F32, kind="ExternalInput")
    outd = nc.dram_tensor("out", [S, D], F32, kind="ExternalOutput")

    with tile.TileContext(nc) as tc:
        with tc.tile_pool(name="persist", bufs=1) as pp, \
             tc.tile_pool(name="dram", bufs=1, space="DRAM") as dp:
            qts = pp.tile([128, TT, S], BF16, tag="qts")
            kts = pp.tile([128, TT, S], BF16, tag="kts")
            vsb = pp.tile([128, ST, 8, HD + 1], BF16, tag="vsb")
            valsT = pp.tile([128, TT, S], BF16, tag="valsT")
            wos = pp.tile([128, TT, D], BF16, tag="wos")
            bqt = pp.tile([128, TT], F32, tag="bqt")

            nc.vector.memset(vsb[:, :, :, HD:HD + 1], 1.0)

            # ---- Phase 1: projections (fp32r, drains to bf16) ----
            with tc.tile_pool(name="proj", bufs=1) as jp, \
                 tc.tile_pool(name="ps_proj", bufs=8, space="PSUM") as psp:
                xts = jp.tile([128, KT_, S], FR, tag="xts")
                wqs = jp.tile([128, KT_, G], FR, tag="wqs")
                wks = jp.tile([128, KT_, G], FR, tag="wks")
                wvs = jp.tile([128, KT_, G], FR, tag="wvs")
                # stagger loads so K-proj never outruns the x stream
                for k in range(KT_):
                    nc.sync.dma_start(out=wks[:, k, :], in_=wkd[k * 128:(k + 1) * 128, :])
                    nc.sync.dma_start(out=xts[:, k, :], in_=xtd[k * 128:(k + 1) * 128, :])
                for k in range(KT_):
                    nc.sync.dma_start(out=wvs[:, k, :], in_=wvd[k * 128:(k + 1) * 128, :])
                for k in range(KT_):
                    nc.sync.dma_start(out=wqs[:, k, :], in_=wqd[k * 128:(k + 1) * 128, :])
                for t in range(TT):
                    nc.sync.dma_start(out=wos[:, t, :], in_=wod[t * 128:(t + 1) * 128, :])
                # bq is host-pretransposed to [128, 4] p-major (16B/partition
                # descriptors); keep this small gather off the sync queue and
                # behind the big loads so it can't head-of-line block them.
                nc.gpsimd.dma_start(
                    out=bqt, in_=bqd.rearrange("(p t) -> p t", t=TT))

                # K^T then Q^T: weight-stationary, 2-t-tile groups, k-outer
                for which, ws, dst in (("k", wks, kts), ("q", wqs, qts)):
                    for tg in range(2):
                        pss = [[psp.tile([128, 512], F32, tag="pj", name="pj")
                                for _ in range(4)] for _ in range(2)]
                        for k in range(KT_):
                            for ti in range(2):
                                t = 2 * tg + ti
                                for sc in range(4):
                                    nc.tensor.matmul(
                                        pss[ti][sc],
                                        ws[:, k, t * 128:(t + 1) * 128],
                                        xts[:, k, sc * 512:(sc + 1) * 512],
                                        start=(k == 0), stop=(k == KT_ - 1))
                        for ti in range(2):
                            t = 2 * tg + ti
                            for sc in range(4):
                                d_ = dst[:, t, sc * 512:(sc + 1) * 512]
                                if which == "q":
                                    nc.vector.tensor_scalar_add(
                                        d_, pss[ti][sc], bqt[:, t:t + 1])
                                else:
                                    nc.vector.tensor_copy(out=d_, in_=pss[ti][sc])

                # V: x-stationary
                for s_ in range(ST):
                    ps = psp.tile([128, 512], F32, tag="pj", name="pj")
                    for k in range(KT_):
                        nc.tensor.matmul(
                            ps, xts[:, k, s_ * 128:(s_ + 1) * 128],
                            wvs[:, k, :],
                            start=(k == 0), stop=(k == KT_ - 1))
                    nc.vector.tensor_copy(
                        out=vsb[:, s_, :, 0:HD],
                        in_=ps.rearrange("p (h d) -> p h d", h=8))

            # ---- Phase 2: attention + output projection ----
            with tc.tile_pool(name="ptp", bufs=4) as ptp, \
                 tc.tile_pool(name="bcp", bufs=2) as bcp, \
                 tc.tile_pool(name="smp", bufs=2) as smp, \
                 tc.tile_pool(name="vshift", bufs=2) as vsp, \
                 tc.tile_pool(name="outp", bufs=2) as op_, \
                 tc.tile_pool(name="ps_lg", bufs=2, space="PSUM") as lgp, \
                 tc.tile_pool(name="ps_pv", bufs=2, space="PSUM") as pvp:
                for blk in range(SB):
                    s0b = blk * SBW
                    for h in range(8):
                        t, p0 = h // 2, (h % 2) * 64
                        qrow = qts[p0:p0 + 64, t, s0b:s0b + SBW]
                        pv = pvp.tile([65, SBW], F32, tag="pv")
                        for sj in range(ST):
                            lg = lgp.tile([128, SBW], F32, tag="lg")
                            lkt = kts[p0:p0 + 64, t, sj * 128:(sj + 1) * 128]
                            for hf in range(2):
                                nc.tensor.matmul(
                                    lg[:, hf * 512:(hf + 1) * 512],
                                    lkt, qrow[:, hf * 512:(hf + 1) * 512],
                                    start=True, stop=True)
                            pt = ptp.tile([128, SBW], BF16, tag="pt")
                            nc.scalar.activation(pt, lg, Exp, scale=0.125)
                            lv = vsb[:, sj, h, 0:HD + 1]
                            for hf in range(2):
                                nc.tensor.matmul(
                                    pv[:, hf * 512:(hf + 1) * 512],
                                    lv, pt[:, hf * 512:(hf + 1) * 512],
                                    start=(sj == 0), stop=(sj == ST - 1))
                        # softmax sums -> reciprocal on 128 partitions
                        # (DMA can't read PSUM: DVE-copy the row out first,
                        # then bounce through DRAM to scatter across partitions)
                        srow_sb = smp.tile([1, SBW], F32, tag="srow_sb")
                        nc.vector.tensor_copy(out=srow_sb, in_=pv[64:65, :])
                        srow = dp.tile([SBW], F32, tag="srow", bufs=3)
                        nc.gpsimd.dma_start(
                            out=srow.rearrange("(a b) -> a b", a=1), in_=srow_sb)
                        ssb = smp.tile([128, SBW // 128], F32, tag="ssb")
                        nc.gpsimd.dma_start(
                            out=ssb, in_=srow.rearrange("(p f) -> p f", p=128))
                        rsb = smp.tile([128, SBW // 128], F32, tag="rsb")
                        nc.vector.reciprocal(out=rsb, in_=ssb)
                        rrow = dp.tile([SBW], F32, tag="rrow", bufs=3)
                        nc.gpsimd.dma_start(
                            out=rrow.rearrange("(p f) -> p f", p=128), in_=rsb)
                        bc = bcp.tile([64, SBW], F32, tag="bc")
                        nc.gpsimd.dma_start(
                            out=bc,
                            in_=rrow.rearrange("(a b) -> a b", a=1)
                                    .partition_broadcast(64))
                        if p0 == 0:
                            nc.vector.tensor_mul(
                                valsT[0:64, t, s0b:s0b + SBW], pv[0:64, :], bc)
                        else:
                            vs = vsp.tile([64, SBW], BF16, tag="vs")
                            nc.vector.tensor_mul(vs, pv[0:64, :], bc)
                            nc.gpsimd.dma_start(
                                out=valsT[64:128, t, s0b:s0b + SBW], in_=vs)

                    # output projection for this si block (reuses lg ring)
                    for st in range(SBW // 128):
                        s0 = s0b + st * 128
                        ops = lgp.tile([128, SBW], F32, tag="lg", name="ops")
                        for gt in range(TT):
                            for hf in range(2):
                                nc.tensor.matmul(
                                    ops[:, hf * 512:(hf + 1) * 512],
                                    valsT[:, gt, s0:s0 + 128],
                                    wos[:, gt, hf * 512:(hf + 1) * 512],
                                    start=(gt == 0), stop=(gt == TT - 1))
                        ob = op_.tile([128, D], F32, tag="ob")
                        nc.vector.tensor_copy(out=ob, in_=ops)
                        nc.gpsimd.dma_start(out=outd[s0:s0 + 128, :], in_=ob)

    _split_sync_waits(nc, mybir)
    return nc


def _get_nc():
    if "nc" not in _cache:
        _cache["nc"] = _build()
    return _cache["nc"]


def _run(in_maps, **kw):
    from concourse.bass_utils import run_bass_kernel_spmd
    return run_bass_kernel_spmd(_get_nc(), in_maps, core_ids=list(range(NCORES)), **kw)


def _make_in_maps(x, Wq, bq, Wk, bk, Wv, bv, Wo, bo):
    import ml_dtypes
    x = np.asarray(x, np.float32)
    Wo32 = np.asarray(Wo, np.float32)
    in_maps = []
    for c in range(NCORES):
        b, g = c // 2, c % 2
        gs = slice(g * G, (g + 1) * G)
        in_maps.append({
            "xt": np.ascontiguousarray(x[b].T),
            "wq": np.ascontiguousarray(np.asarray(Wq, np.float32)[:, gs]),
            "wk": np.ascontiguousarray(np.asarray(Wk, np.float32)[:, gs]),
            "wv": np.ascontiguousarray(np.asarray(Wv, np.float32)[:, gs]),
            "wo": np.ascontiguousarray(Wo32[gs, :].astype(ml_dtypes.bfloat16)),
            # pre-transposed to [128 partitions, 4 t-tiles] p-major so the
            # on-chip gather is 16B-per-partition descriptors, not 4B
            "bq": np.ascontiguousarray(
                np.asarray(bq, np.float32)[gs].reshape(TT, 128).T),
        })
    return in_maps


def kernel(x, Wq, bq, Wk, bk, Wv, bv, Wo, bo, **_kw):
    res = _run(_make_in_maps(x, Wq, bq, Wk, bk, Wv, bv, Wo, bo))
    # host-side bias: bv @ Wo + bo (exact: softmax rows sum to 1)
    hb = (np.asarray(bv, np.float64) @ np.asarray(Wo, np.float64)
          + np.asarray(bo, np.float64)).astype(np.float32)
    out = np.empty((B, S, D), dtype=np.float32)
    for b in range(B):
        out[b] = res.results[2 * b]["out"] + res.results[2 * b + 1]["out"] + hb
    return out
